# revision 2
# baseline (speedup 1.0000x reference)
"""MoE layer (B=4, T=2048, C=1024, F=4096, E=8, top-2) on 8 trn2 NeuronCores.

Strategy: 8-way tensor parallelism over the FFN width F (not expert
parallelism).  The gate + top-2 routing runs on the host; tokens are
gathered into per-expert segments (total Sum n_e = N*topk = 16384
token-expert pairs).  EVERY core processes ALL 16384 pairs, but only a
1/8 slice of F (F_local = 512) of every expert:

    layer1:  h_loc = gelu(x @ w1[e][:, c*512:(c+1)*512] + b1_loc)
    layer2:  y_part = h_loc @ w2[e][c*512:(c+1)*512, :]      (no bias)

The host sums the 8 partial y's, adds b2, applies the top-2 combine
weights and scatter-adds into the full output.  Because all cores run
the exact same token segments, the SPMD program is naturally
shape-uniform with ZERO padding: per-core work = 16384 * 64 PE cycles
= 437 us at the 78.6 TF/s bf16 roofline, independent of the expert
load imbalance (max load 2129 vs mean 2048 would cost 4% in the
expert-parallel layout).

Device layout (weights stationary, tokens stream as moving operand):
  xt   [KC=8, 128, NTOK]    bf16  all routed tokens, transposed, segment-
                                  concatenated (same for every core)
  w1l  [E*KC, 128, 512]     bf16  w1[e][kc-slice, local F cols]
  b1l  [128, E*4]           f32   local b1 transposed (partition = F%128)
  w2l  [E*4, 128, C]        bf16  w2[e][local F rows, :] (partition = F%128)
  yt   [KC=8, 128, NTOK]    bf16  partial y, transposed
"""

import numpy as np
import ml_dtypes

B, T, C, F, E, TOPK = 4, 2048, 1024, 4096, 8, 2
N_CORES = 8
KC = C // 128          # 8  C-slices (layer-1 contraction / layer-2 output)
FL = F // N_CORES      # 512 local F columns per core
KFL = FL // 128        # 4  local F-slices
TOK_TILE = 512

_BF16 = ml_dtypes.bfloat16

_nc_cache: dict[tuple, object] = {}


def _token_tiles(cap: int):
    """Split cap into equal-ish tiles of at most TOK_TILE tokens.

    Equal sizes keep every matmul's streaming time above the LDWEIGHTS
    shadow (a small tail tile would be weight-load-bound on the PE)."""
    n = -(-cap // TOK_TILE)
    base, rem = divmod(cap, n)
    tiles, off = [], 0
    for i in range(n):
        t = base + (1 if i < rem else 0)
        tiles.append((off, t))
        off += t
    return tiles


def build_moe_nc(n_toks: tuple, act: str = "Gelu"):
    """Build + compile the per-core Bass program.

    n_toks[e] = number of tokens routed to expert e (same on all cores;
    every core sees every token, sliced along F)."""
    import concourse.mybir as mybir
    import concourse.tile as tile
    from concourse import bacc

    dt = mybir.dt
    GELU = getattr(mybir.ActivationFunctionType, act)
    IDENT = mybir.ActivationFunctionType.Identity

    ntok = int(sum(n_toks))

    nc = bacc.Bacc("TRN2", target_bir_lowering=False, debug=False)

    xt_d = nc.dram_tensor("xt", [KC, 128, ntok], dt.bfloat16, kind="ExternalInput")
    w1_d = nc.dram_tensor("w1l", [E * KC, 128, FL], dt.bfloat16, kind="ExternalInput")
    b1_d = nc.dram_tensor("b1l", [128, E * KFL], dt.float32, kind="ExternalInput")
    w2_d = nc.dram_tensor("w2l", [E * KFL, 128, C], dt.bfloat16, kind="ExternalInput")
    yt_d = nc.dram_tensor("yt", [KC, 128, ntok], dt.bfloat16, kind="ExternalOutput")

    with tile.TileContext(nc) as tc:
        with (
            tc.tile_pool(name="wpool", bufs=1) as wpool,
            tc.tile_pool(name="xpool", bufs=3) as xpool,
            tc.tile_pool(name="hpool", bufs=2) as hpool,
            tc.tile_pool(name="ypool", bufs=4) as ypool,
            tc.tile_pool(name="pp", bufs=8, space="PSUM") as pp,
        ):
            w1_s = [[None] * KC for _ in range(E)]
            w2_s = [[None] * KFL for _ in range(E)]

            def load_w1(e):
                for kc in range(KC):
                    w = wpool.tile([128, FL], dt.bfloat16, tag=f"w1_{e}_{kc}")
                    nc.sync.dma_start(w[:], w1_d[e * KC + kc, :, :])
                    w1_s[e][kc] = w

            def load_w2(e):
                for kf in range(KFL):
                    w = wpool.tile([128, C], dt.bfloat16, tag=f"w2_{e}_{kf}")
                    nc.sync.dma_start(w[:], w2_d[e * KFL + kf, :, :])
                    w2_s[e][kf] = w

            def load_xt(goff, tsz):
                # one tile per C-slice so each matmul only waits on the
                # 128-partition slab it actually streams
                xt_s = []
                for kc in range(KC):
                    xk = xpool.tile([128, tsz], dt.bfloat16, tag=f"xt_{kc}")
                    nc.sync.dma_start(xk[:], xt_d[kc, :, goff : goff + tsz])
                    xt_s.append(xk)
                return xt_s

            # DMA priority order = consumption order: expert 0's w1 and
            # the first token tile unblock the PE ~2 MB in; then biases;
            # then (w2[e], w1[e+1]) pairs matching the demand schedule
            # (w2[e] is first read mid-segment e, w1[e+1] at its end).
            seg_off = [0]
            for e in range(E):
                seg_off.append(seg_off[-1] + int(n_toks[e]))
            first_tiles = _token_tiles(int(n_toks[0])) if n_toks[0] else []

            load_w1(0)
            xt0_s = load_xt(0, first_tiles[0][1]) if first_tiles else None
            b1_s = wpool.tile([128, E * KFL], dt.float32, tag="b1")
            nc.sync.dma_start(b1_s[:], b1_d[:])
            load_w2(0)
            for e in range(1, E):
                load_w1(e)
                load_w2(e)

            for e in range(E):
                n_e = int(n_toks[e])
                if n_e == 0:
                    continue
                tiles = _token_tiles(n_e)
                for ti, (off, tsz) in enumerate(tiles):
                    goff = seg_off[e] + off
                    xt_s = xt0_s if (e == 0 and ti == 0) else load_xt(goff, tsz)

                    # layer 1: h^T[f_blk, tok] = gelu(w1l^T @ x^T + b1l)
                    ht_s = hpool.tile([128, KFL, tsz], dt.bfloat16, tag="ht")
                    for mf in range(KFL):
                        ps = pp.tile([128, tsz], dt.float32, tag="ps")
                        for kc in range(KC):
                            nc.tensor.matmul(
                                ps[:],
                                w1_s[e][kc][:, mf * 128 : (mf + 1) * 128],
                                xt_s[kc][:],
                                start=(kc == 0),
                                stop=(kc == KC - 1),
                            )
                        nc.scalar.activation(
                            ht_s[:, mf, :], ps[:], GELU,
                            bias=b1_s[:, e * KFL + mf : e * KFL + mf + 1],
                        )

                    # layer 2: y_part^T[c_blk, tok] = w2l^T @ h^T  (bias on host)
                    for mc in range(KC):
                        ps2 = pp.tile([128, tsz], dt.float32, tag="ps")
                        for kf in range(KFL):
                            nc.tensor.matmul(
                                ps2[:],
                                w2_s[e][kf][:, mc * 128 : (mc + 1) * 128],
                                ht_s[:, kf, :],
                                start=(kf == 0),
                                stop=(kf == KFL - 1),
                            )
                        y_s = ypool.tile([128, tsz], dt.bfloat16, tag="y")
                        nc.scalar.activation(y_s[:], ps2[:], IDENT)
                        nc.sync.dma_start(yt_d[mc, :, goff : goff + tsz], y_s[:])

    nc.compile()
    return nc


def _route(x_flat, gate_w, gate_b):
    """Replicates reference gating: softmax -> top-2 -> renormalize."""
    logits = x_flat @ gate_w + gate_b  # [N, E] f32
    m = logits.max(-1, keepdims=True)
    p = np.exp(logits - m)
    p /= p.sum(-1, keepdims=True)
    # jax.lax.top_k: descending, ties -> lower index. Stable argsort matches.
    order = np.argsort(-p, axis=1, kind="stable")[:, :TOPK]  # [N, 2]
    top = np.take_along_axis(p, order, axis=1)
    wts = top / top.sum(-1, keepdims=True)
    return order, wts.astype(np.float32)


def run_moe(inputs: dict, trace: bool = False):
    """Returns (full_output [B,T,C] f32, BassKernelResults)."""
    from concourse.bass_utils import run_bass_kernel_spmd

    x = np.asarray(inputs["x"], dtype=np.float32)
    gate_w = np.asarray(inputs["gate_w"], dtype=np.float32)
    gate_b = np.asarray(inputs["gate_b"], dtype=np.float32)
    w1 = np.asarray(inputs["w1"], dtype=np.float32)
    b1 = np.asarray(inputs["b1"], dtype=np.float32)
    w2 = np.asarray(inputs["w2"], dtype=np.float32)
    b2 = np.asarray(inputs["b2"], dtype=np.float32)

    xf = x.reshape(-1, C)
    order, wts = _route(xf, gate_w, gate_b)

    idx = []
    comb = []
    for e in range(E):
        mask = order == e  # [N, 2]
        rows = np.nonzero(mask.any(axis=1))[0]
        idx.append(rows)
        comb.append((wts[rows] * mask[rows]).sum(axis=1).astype(np.float32))
    n_toks = tuple(len(r) for r in idx)
    ntok = int(sum(n_toks))

    if n_toks not in _nc_cache:
        _nc_cache[n_toks] = build_moe_nc(n_toks)
    nc = _nc_cache[n_toks]

    # xt: all segments concatenated, transposed — identical on every core
    xt = np.empty((C, ntok), dtype=_BF16)
    off = 0
    for e in range(E):
        xt[:, off : off + n_toks[e]] = xf[idx[e]].T
        off += n_toks[e]
    xt = np.ascontiguousarray(xt).reshape(KC, 128, ntok)

    w1b = w1.astype(_BF16)  # [E, C, F]
    w2b = w2.astype(_BF16)  # [E, F, C]

    in_maps = []
    for c in range(N_CORES):
        lo, hi = c * FL, (c + 1) * FL
        w1l = np.ascontiguousarray(w1b[:, :, lo:hi]).reshape(E * KC, 128, FL)
        w2l = np.ascontiguousarray(w2b[:, lo:hi, :]).reshape(E * KFL, 128, C)
        b1l = np.ascontiguousarray(
            b1[:, lo:hi].reshape(E * KFL, 128).T.astype(np.float32)
        )
        in_maps.append({"xt": xt, "w1l": w1l, "b1l": b1l, "w2l": w2l})

    res = run_bass_kernel_spmd(nc, in_maps, list(range(N_CORES)), trace=trace)

    # host combine: sum the 8 partial y's, add b2, apply combine weights
    ysum = np.zeros((C, ntok), dtype=np.float32)
    for c in range(N_CORES):
        ysum += res.results[c]["yt"].reshape(C, ntok).astype(np.float32)

    out = np.zeros_like(xf)
    off = 0
    for e in range(E):
        n_e = n_toks[e]
        if n_e == 0:
            continue
        y = ysum[:, off : off + n_e].T + b2[e]  # [n_e, C]
        out[idx[e]] += comb[e][:, None] * y
        off += n_e
    return out.reshape(B, T, C), res


def kernel(x, gate_w, gate_b, w1, b1, w2, b2):
    out, _ = run_moe(
        {
            "x": x,
            "gate_w": gate_w,
            "gate_b": gate_b,
            "w1": w1,
            "b1": b1,
            "w2": w2,
            "b2": b2,
        }
    )
    return out


# revision 4
# speedup vs baseline: 1.1158x; 1.1158x over previous
"""MoE layer (B=4, T=2048, C=1024, F=4096, E=8, top-2) on 8 trn2 NeuronCores.

Strategy: 8-way tensor parallelism over the FFN width F (not expert
parallelism).  The gate + top-2 routing runs on the host; tokens are
gathered into per-expert segments (total Sum n_e = N*topk = 16384
token-expert pairs).  EVERY core processes ALL 16384 pairs, but only a
1/8 slice of F (F_local = 512) of every expert:

    layer1:  h_loc = gelu(x @ w1[e][:, c*512:(c+1)*512] + b1_loc)
    layer2:  y_part = h_loc @ w2[e][c*512:(c+1)*512, :]      (no bias)

The host sums the 8 partial y's, adds b2, applies the top-2 combine
weights and scatter-adds into the full output.  Because all cores run
the exact same token segments, the SPMD program is naturally
shape-uniform with ZERO padding: per-core work = 16384 * 64 PE cycles
= 437 us at the 78.6 TF/s bf16 roofline, independent of expert load
imbalance.

Perf-critical structure (from trace analysis):
 - Every dma_start costs ~625 ns on the shared HWDGE descriptor
   generator regardless of size, so DMAs are BATCHED: one 3D-AP DMA
   per token tile / output tile / weight block (~90 DMAs total).  DRAM
   layouts are partition-major ([128, ...]) so a single DMA matches
   the SBUF tile layout.
 - Weights stream just-in-time (2 experts ahead) so output DMAs never
   queue behind a long weight prefetch.
 - Software pipelining: layer1 of tile t+1 is emitted before layer2 of
   tile t, hiding the ~600 ns GELU latency of the last h-block (layer2
   only has a 4-deep contraction here, too short to hide it alone).
 - A small first tile (128 tokens) gets the PE started while the DMA
   stream ramps.

Device layout (weights stationary, tokens stream as moving operand):
  xt   [128, KC, NTOK]      bf16  all routed tokens, transposed
  w1l  [128, E, KC, FL]     bf16  w1[e][kc-slice, local F cols]
  b1l  [128, E*KFL]         f32   local b1 (partition = F%128)
  w2l  [128, E, KFL, C]     bf16  w2[e][local F rows, :]
  yt   [128, KC, NTOK]      bf16  partial y, transposed
"""

import numpy as np
import ml_dtypes

B, T, C, F, E, TOPK = 4, 2048, 1024, 4096, 8, 2
N_CORES = 8
KC = C // 128          # 8  C-slices (layer-1 contraction / layer-2 output)
FL = F // N_CORES      # 512 local F columns per core
KFL = FL // 128        # 4  local F-slices
TOK_TILE = 512
TILE0 = 128            # small starter tile while the DMA stream ramps

_BF16 = ml_dtypes.bfloat16

_nc_cache: dict[tuple, object] = {}


def _token_tiles(cap: int, first_small: bool):
    """Split cap into equal-ish tiles of at most TOK_TILE tokens.

    Equal sizes keep every matmul's streaming time above the LDWEIGHTS
    shadow (a small tail tile would be weight-load-bound on the PE)."""
    tiles = []
    off = 0
    if first_small and cap > 2 * TILE0:
        tiles.append((0, TILE0))
        off = TILE0
        cap -= TILE0
    n = -(-cap // TOK_TILE)
    base, rem = divmod(cap, n)
    for i in range(n):
        t = base + (1 if i < rem else 0)
        tiles.append((off, t))
        off += t
    return tiles


def build_moe_nc(n_toks: tuple, act: str = "Gelu"):
    """Build + compile the per-core Bass program.

    n_toks[e] = number of tokens routed to expert e (same on all cores;
    every core sees every token, sliced along F)."""
    import concourse.mybir as mybir
    import concourse.tile as tile
    from concourse import bacc

    dt = mybir.dt
    GELU = getattr(mybir.ActivationFunctionType, act)
    IDENT = mybir.ActivationFunctionType.Identity

    ntok = int(sum(n_toks))

    nc = bacc.Bacc("TRN2", target_bir_lowering=False, debug=False)

    xt_d = nc.dram_tensor("xt", [128, KC, ntok], dt.bfloat16, kind="ExternalInput")
    w1_d = nc.dram_tensor("w1l", [128, E, KC, FL], dt.bfloat16, kind="ExternalInput")
    b1_d = nc.dram_tensor("b1l", [128, E * KFL], dt.float32, kind="ExternalInput")
    w2_d = nc.dram_tensor("w2l", [128, E, KFL, C], dt.bfloat16, kind="ExternalInput")
    yt_d = nc.dram_tensor("yt", [128, KC, ntok], dt.bfloat16, kind="ExternalOutput")

    # global tile list: (expert, global token offset, size)
    seg_off = [0]
    for e in range(E):
        seg_off.append(seg_off[-1] + int(n_toks[e]))
    all_tiles = []
    for e in range(E):
        if n_toks[e] == 0:
            continue
        for off, tsz in _token_tiles(int(n_toks[e]), first_small=(e == 0)):
            all_tiles.append((e, seg_off[e] + off, tsz))
    n_tiles = len(all_tiles)
    # first tile index of each expert (where to JIT-load weights)
    first_tile_of = {}
    for i, (e, _, _) in enumerate(all_tiles):
        first_tile_of.setdefault(e, i)

    with tile.TileContext(nc) as tc:
        with (
            tc.tile_pool(name="wpool", bufs=1) as wpool,
            tc.tile_pool(name="xpool", bufs=3) as xpool,
            tc.tile_pool(name="hpool", bufs=2) as hpool,
            tc.tile_pool(name="ypool", bufs=2) as ypool,
            tc.tile_pool(name="pp", bufs=8, space="PSUM") as pp,
        ):
            w1_s: list = [None] * E   # expert 0: [half0, half1]; else tile
            w2_s: list = [None] * E

            def load_w1(e):
                w = wpool.tile([128, KC, FL], dt.bfloat16, tag=f"w1_{e}")
                nc.sync.dma_start(w[:], w1_d[:, e, :, :])
                w1_s[e] = w

            def load_w2(e):
                w = wpool.tile([128, KFL, C], dt.bfloat16, tag=f"w2_{e}")
                nc.sync.dma_start(w[:], w2_d[:, e, :, :])
                w2_s[e] = w

            def load_xt(t):
                _, goff, tsz = all_tiles[t]
                xk = xpool.tile([128, KC, tsz], dt.bfloat16, tag="xt")
                nc.sync.dma_start(xk[:], xt_d[:, :, goff : goff + tsz])
                return xk

            # --- prefetch: minimal set to unblock the PE, then JIT ---
            b1_s = wpool.tile([128, E * KFL], dt.float32, tag="b1")
            nc.sync.dma_start(b1_s[:], b1_d[:])
            # expert 0 w1 in two kc-halves so the starter tile can begin
            # after half the weights
            w1e0 = []
            for h in range(2):
                w = wpool.tile([128, KC // 2, FL], dt.bfloat16, tag=f"w1_0_{h}")
                nc.sync.dma_start(
                    w[:], w1_d[:, 0, h * (KC // 2) : (h + 1) * (KC // 2), :]
                )
                w1e0.append(w)
            w1_s[0] = w1e0
            xt_tiles: dict[int, object] = {0: load_xt(0)}
            load_w2(0)
            if E > 1:
                load_w1(1)
                load_w2(1)

            def w1_ap(e, kc, mf):
                if e == 0:
                    h, r = divmod(kc, KC // 2)
                    return w1_s[0][h][:, r, mf * 128 : (mf + 1) * 128]
                return w1_s[e][:, kc, mf * 128 : (mf + 1) * 128]

            ht_tiles: dict[int, object] = {}

            def emit_L1(t):
                e, _, tsz = all_tiles[t]
                xt_s = xt_tiles.pop(t)
                ht_s = hpool.tile([128, KFL, tsz], dt.bfloat16, tag="ht")
                ht_tiles[t] = ht_s
                if t == 0:
                    # kc-half-outer accumulation: start on the first w1
                    # half-DMA while the second streams in
                    ps_w = [
                        pp.tile([128, tsz], dt.float32, tag="ps", name=f"ps0_{i}")
                        for i in range(KFL)
                    ]
                    for kc in range(KC):
                        for mf in range(KFL):
                            nc.tensor.matmul(
                                ps_w[mf][:], w1_ap(e, kc, mf), xt_s[:, kc, :],
                                start=(kc == 0), stop=(kc == KC - 1),
                            )
                    for mf in range(KFL):
                        nc.scalar.activation(
                            ht_s[:, mf, :], ps_w[mf][:], GELU,
                            bias=b1_s[:, e * KFL + mf : e * KFL + mf + 1],
                        )
                    return
                for mf in range(KFL):
                    ps = pp.tile([128, tsz], dt.float32, tag="ps")
                    for kc in range(KC):
                        nc.tensor.matmul(
                            ps[:], w1_ap(e, kc, mf), xt_s[:, kc, :],
                            start=(kc == 0), stop=(kc == KC - 1),
                        )
                    nc.scalar.activation(
                        ht_s[:, mf, :], ps[:], GELU,
                        bias=b1_s[:, e * KFL + mf : e * KFL + mf + 1],
                    )

            def emit_L2(t):
                e, goff, tsz = all_tiles[t]
                ht_s = ht_tiles.pop(t)
                y_s = ypool.tile([128, KC, tsz], dt.bfloat16, tag="y")
                for mc in range(KC):
                    ps2 = pp.tile([128, tsz], dt.float32, tag="ps")
                    for kf in range(KFL):
                        nc.tensor.matmul(
                            ps2[:],
                            w2_s[e][:, kf, mc * 128 : (mc + 1) * 128],
                            ht_s[:, kf, :],
                            start=(kf == 0), stop=(kf == KFL - 1),
                        )
                    nc.scalar.activation(y_s[:, mc, :], ps2[:], IDENT)
                nc.sync.dma_start(yt_d[:, :, goff : goff + tsz], y_s[:])

            # --- software-pipelined main loop: L1 runs one tile ahead ---
            emit_L1(0)
            for t in range(n_tiles):
                if t + 1 < n_tiles:
                    e_next = all_tiles[t + 1][0]
                    if t + 1 == first_tile_of.get(e_next) and e_next + 1 < E:
                        if w1_s[e_next + 1] is None:
                            load_w1(e_next + 1)
                            load_w2(e_next + 1)
                    xt_tiles[t + 1] = load_xt(t + 1)
                    emit_L1(t + 1)
                emit_L2(t)

    nc.compile()
    return nc


def _route(x_flat, gate_w, gate_b):
    """Replicates reference gating: softmax -> top-2 -> renormalize."""
    logits = x_flat @ gate_w + gate_b  # [N, E] f32
    m = logits.max(-1, keepdims=True)
    p = np.exp(logits - m)
    p /= p.sum(-1, keepdims=True)
    # jax.lax.top_k: descending, ties -> lower index. Stable argsort matches.
    order = np.argsort(-p, axis=1, kind="stable")[:, :TOPK]  # [N, 2]
    top = np.take_along_axis(p, order, axis=1)
    wts = top / top.sum(-1, keepdims=True)
    return order, wts.astype(np.float32)


def run_moe(inputs: dict, trace: bool = False):
    """Returns (full_output [B,T,C] f32, BassKernelResults)."""
    from concourse.bass_utils import run_bass_kernel_spmd

    x = np.asarray(inputs["x"], dtype=np.float32)
    gate_w = np.asarray(inputs["gate_w"], dtype=np.float32)
    gate_b = np.asarray(inputs["gate_b"], dtype=np.float32)
    w1 = np.asarray(inputs["w1"], dtype=np.float32)
    b1 = np.asarray(inputs["b1"], dtype=np.float32)
    w2 = np.asarray(inputs["w2"], dtype=np.float32)
    b2 = np.asarray(inputs["b2"], dtype=np.float32)

    xf = x.reshape(-1, C)
    order, wts = _route(xf, gate_w, gate_b)

    idx = []
    comb = []
    for e in range(E):
        mask = order == e  # [N, 2]
        rows = np.nonzero(mask.any(axis=1))[0]
        idx.append(rows)
        comb.append((wts[rows] * mask[rows]).sum(axis=1).astype(np.float32))
    n_toks = tuple(len(r) for r in idx)
    ntok = int(sum(n_toks))

    if n_toks not in _nc_cache:
        _nc_cache[n_toks] = build_moe_nc(n_toks)
    nc = _nc_cache[n_toks]

    # xt: all segments concatenated, transposed — identical on every core
    xcat = np.empty((ntok, C), dtype=np.float32)
    off = 0
    for e in range(E):
        xcat[off : off + n_toks[e]] = xf[idx[e]]
        off += n_toks[e]
    # [ntok, C] -> [128, KC, ntok]  (partition-major for single-DMA tiles)
    xt = np.ascontiguousarray(
        xcat.T.reshape(KC, 128, ntok).transpose(1, 0, 2).astype(_BF16)
    )

    w1b = w1.astype(_BF16)  # [E, C, F]
    w2b = w2.astype(_BF16)  # [E, F, C]

    in_maps = []
    for c in range(N_CORES):
        lo, hi = c * FL, (c + 1) * FL
        w1l = np.ascontiguousarray(
            w1b[:, :, lo:hi].reshape(E, KC, 128, FL).transpose(2, 0, 1, 3)
        )
        w2l = np.ascontiguousarray(
            w2b[:, lo:hi, :].reshape(E, KFL, 128, C).transpose(2, 0, 1, 3)
        )
        b1l = np.ascontiguousarray(
            b1[:, lo:hi].reshape(E * KFL, 128).T.astype(np.float32)
        )
        in_maps.append({"xt": xt, "w1l": w1l, "b1l": b1l, "w2l": w2l})

    res = run_bass_kernel_spmd(nc, in_maps, list(range(N_CORES)), trace=trace)

    # host combine: sum the 8 partial y's, add b2, apply combine weights
    ysum = np.zeros((128, KC, ntok), dtype=np.float32)
    for c in range(N_CORES):
        ysum += res.results[c]["yt"]
    ysum = ysum.transpose(1, 0, 2).reshape(C, ntok)

    out = np.zeros_like(xf)
    off = 0
    for e in range(E):
        n_e = n_toks[e]
        if n_e == 0:
            continue
        y = ysum[:, off : off + n_e].T + b2[e]  # [n_e, C]
        out[idx[e]] += comb[e][:, None] * y
        off += n_e
    return out.reshape(B, T, C), res


def kernel(x, gate_w, gate_b, w1, b1, w2, b2):
    out, _ = run_moe(
        {
            "x": x,
            "gate_w": gate_w,
            "gate_b": gate_b,
            "w1": w1,
            "b1": b1,
            "w2": w2,
            "b2": b2,
        }
    )
    return out
